# revision 21
# baseline (speedup 1.0000x reference)
"""Trainium2 Bass kernel for nn_BDFM_46428596469849.

Per-batch math (B=8, C=256, H=W=128, HW=16384):
    m   = relu(m); z = (m > 0.3)
    er  = minpool4x4(z, SAME, border=1); di = maxpool4x4(z, SAME, border=0)
    fbu = [er, 1-di, di-er]                          # [3, HW]
    mid = fbu @ F^T                                  # [3, C]
    cf  = bn_f(Wf @ F);  mid1 = mid @ cf;  mid2 = mid^T @ mid1
    out = bn_o(W_out @ [F; mid2])

The chain collapses algebraically: with sf/bf (resp. so/bo) the BN scale/bias,
    g    = mid @ [diag(sf) Wf | bf]       # [3, C+1] (col C is u = mid @ bf)
    S    = mid @ W2^T                     # [3, C]   (W_out = [W1 | W2])
    WeffT= W1^T + g^T @ S                 # [C, C]   (= (W1 + W2 @ mid^T g)^T)
    wv   = S^T @ u                        # [C]      (= W2 @ mid^T @ u)
    out  = diag(so) @ WeffT^T @ F + (so*wv + bo) 1^T
so each batch element needs: the midT reduction (one pass over F with PE
transposes), a three-stage tiny-algebra chain, and one C x C x HW matmul
streamed over resident F.

Sharding: data-parallel, one batch element per NeuronCore (8 cores).
"""

import os
import sys

for _p in ("/opt/trn_rl_repo", "/root/.axon_site/_ro/trn_rl_repo"):
    if os.path.isdir(_p) and _p not in sys.path:
        sys.path.insert(0, _p)

import numpy as np

import concourse.bass as bass
import concourse.mybir as mybir
import concourse.tile as tile
from concourse.bass_utils import run_bass_kernel_spmd
from concourse.masks import make_identity

dt = mybir.dt
AF = mybir.ActivationFunctionType
OP = mybir.AluOpType

B, C, H, W = 8, 256, 128, 128
HW = H * W
NCORES = 8
EPS = 1e-5
F32R = dt.float32r


def _split_drain_waits(nc, max_waits=1):
    # Walrus codegen rejects instructions carrying more than a couple of
    # semaphore waits (CTRL drains and DMA descriptors in particular). Hoist
    # excess waits onto preceding NoOps on the same engine queue — the queue
    # executes in order, so the waits are satisfied before the instruction.
    for f in nc.m.functions:
        for bb in f.blocks:
            new_insts = []
            for inst in bb.instructions:
                si = inst.sync_info
                if si is not None and si.on_wait and len(si.on_wait) > max_waits:
                    waits = list(si.on_wait)
                    while len(waits) > max_waits:
                        chunk, waits = waits[:max_waits], waits[max_waits:]
                        pre = mybir.InstNoOp(
                            name=f"I-wsplit-{nc.next_id()}",
                            engine=inst.engine,
                            sync_info=mybir.SyncInfo(on_wait=chunk, on_update=[]),
                        )
                        nc.inst_map[pre.name] = pre
                        new_insts.append(pre)
                    inst.sync_info = mybir.SyncInfo(
                        on_wait=waits, on_update=list(si.on_update)
                    )
                new_insts.append(inst)
            bb.instructions[:] = new_insts


def build_nc():
    from contextlib import ExitStack

    nc = bass.Bass("TRN2", target_bir_lowering=False)

    feat = nc.declare_dram_parameter("feature", [C, HW], dt.float32, isOutput=False)
    m_in = nc.declare_dram_parameter("m", [H, W], dt.float32, isOutput=False)
    wfeat = nc.declare_dram_parameter("w_feat", [C, C], dt.float32, isOutput=False)
    wout = nc.declare_dram_parameter("w_out", [C, 2 * C], dt.float32, isOutput=False)
    bnp = {}
    for pre in ("f", "o"):
        for nm in ("gamma", "beta", "mean", "var"):
            key = f"bn_{pre}_{nm}"
            bnp[key] = nc.declare_dram_parameter(key, [C], dt.float32, isOutput=False)
    out_d = nc.declare_dram_parameter("out", [C, HW], dt.float32, isOutput=True)

    with tile.TileContext(nc) as tc, ExitStack() as ctx:
        const = ctx.enter_context(tc.tile_pool(name="const", bufs=1))
        ident = const.tile([128, 128], dt.float32, name="ident")
        make_identity(nc, ident)
        ident_r = const.tile([128, 128], F32R, name="ident_r")
        nc.vector.tensor_copy(ident_r, ident)
        eps_t = const.tile([2, 1], dt.float32, name="eps_t")
        nc.vector.memset(eps_t, EPS)

        # ---- resident feature in float32r, as per-piece tiles ----
        # fp32 DRAM bits land directly as f32r via bitcast; separate per-piece
        # tiles keep the dependency granularity fine so the midT phase
        # overlaps the load. 2048-col pieces = 1 MB / 8 KB-per-partition DMAs.
        NPIECE = 8
        PIECE = HW // NPIECE
        fpool = ctx.enter_context(tc.tile_pool(name="fpool", bufs=1))
        F_t = [
            [
                fpool.tile([128, PIECE], F32R, name=f"F{cc}_{i}", tag=f"F{cc}_{i}")
                for i in range(NPIECE)
            ]
            for cc in range(2)
        ]

        def f_slice(cc, col0, width):
            i = col0 // PIECE
            off = col0 % PIECE
            assert off + width <= PIECE
            return F_t[cc][i][:, off : off + width]

        # ---- ACT activation-table preload (Sqrt), off the critical path ----
        act_warm = const.tile([2, 1], dt.float32, name="act_warm")
        nc.scalar.activation(out=act_warm, in_=eps_t, func=AF.Sqrt, bias=eps_t, scale=1.0)

        # ---- m first on the sync ring (morphology is on the critical path) --
        m_sb = const.tile([128, 128], dt.float32, name="m_sb")
        nc.sync.dma_start(out=m_sb, in_=m_in[:, :])

        # ---- feature loads: even pieces on the sync HWDGE ring, odd pieces
        # via the gpsimd SWDGE queue. The scalar(ACT) ring carries NO loads —
        # the HWDGE ring depth (~2) would block the ACT sequencer, and ACT
        # must stay free for the f1T evictions. ----
        for i in range(NPIECE):
            eng = nc.sync if i % 2 == 0 else nc.gpsimd
            for cc in range(2):
                eng.dma_start(
                    out=F_t[cc][i][:],
                    in_=feat[
                        cc * 128 : (cc + 1) * 128, i * PIECE : (i + 1) * PIECE
                    ].bitcast(F32R),
                )

        # ---- small inputs, behind the odd F pieces on the gpsimd queue ----
        wf = []
        wo = []
        for oc in range(2):
            t = const.tile([128, C], dt.float32, name=f"wf{oc}", tag=f"wf{oc}")
            nc.gpsimd.dma_start(out=t, in_=wfeat[oc * 128 : (oc + 1) * 128, :])
            wf.append(t)
            t2 = const.tile([128, 2 * C], dt.float32, name=f"wo{oc}", tag=f"wo{oc}")
            nc.gpsimd.dma_start(out=t2, in_=wout[oc * 128 : (oc + 1) * 128, :])
            wo.append(t2)

        # BN params as [2, 128] (2 contiguous 512B descriptors each, instead
        # of 256 4-byte gather descriptors in [128, 2] layout).
        bnt = {}
        for key, hdl in bnp.items():
            t = const.tile([2, 128], dt.float32, name=f"t_{key}", tag=f"t_{key}")
            nc.gpsimd.dma_start(out=t, in_=hdl[:].rearrange("(t p) -> t p", p=128))
            bnt[key] = t

        # ---- morphology: separable 4x4 window (offsets -1..+2), both passes
        # along the free dim with a PE transpose in between; border = the
        # reduction identity (matches reduce_window SAME + init value) ----
        mor = ctx.enter_context(tc.tile_pool(name="mor", bufs=1))

        def pool1d_free(eng, src, op, border, label):
            padd = mor.tile([128, 131], dt.float32, name=f"pad_{label}", tag=f"pad_{label}")
            eng.memset(padd, border)
            eng.tensor_copy(padd[:, 1:129], src)
            a = mor.tile([128, 130], dt.float32, name=f"a_{label}", tag=f"a_{label}")
            eng.tensor_tensor(a, padd[:, 0:130], padd[:, 1:131], op)
            r = mor.tile([128, 128], dt.float32, name=f"r_{label}", tag=f"r_{label}")
            eng.tensor_tensor(r, a[:, 0:128], a[:, 2:130], op)
            return r

        z = mor.tile([128, 128], dt.float32, name="z")
        nc.vector.tensor_scalar(out=z, in0=m_sb, scalar1=0.3, scalar2=None, op0=OP.is_gt)
        erw = pool1d_free(nc.vector, z, OP.min, 1.0, "er1")  # [h, w] pooled over w
        diw = pool1d_free(nc.vector, z, OP.max, 0.0, "di1")
        with tc.tile_pool(name="mor_ps", bufs=1, space="PSUM") as mor_ps:
            er_ps = mor_ps.tile([128, 128], dt.float32, name="er_ps", tag="er_ps")
            nc.tensor.transpose(er_ps, erw, ident)
            erwT = mor.tile([128, 128], dt.float32, name="erwT")
            nc.vector.tensor_copy(erwT, er_ps)
            di_ps = mor_ps.tile([128, 128], dt.float32, name="di_ps", tag="di_ps")
            nc.tensor.transpose(di_ps, diw, ident)
            diwT = mor.tile([128, 128], dt.float32, name="diwT")
            nc.vector.tensor_copy(diwT, di_ps)
        erT = pool1d_free(nc.vector, erwT, OP.min, 1.0, "er2")  # [w, h] pooled over h
        diT = pool1d_free(nc.vector, diwT, OP.max, 0.0, "di2")

        # fbuT[w, h, k] = fbu[k, h*128+w]
        fbuT = mor.tile([128, 128, 3], F32R, name="fbuT")
        nc.vector.tensor_copy(fbuT[:, :, 0], erT)
        nc.vector.tensor_scalar(
            out=fbuT[:, :, 1], in0=diT, scalar1=-1.0, scalar2=1.0, op0=OP.mult, op1=OP.add
        )
        nc.vector.tensor_tensor(fbuT[:, :, 2], diT, erT, OP.subtract)

        # ---- BN scale/bias in [2, 128] layout, then one packed PE transpose:
        # s = gamma*rsqrt(var+eps), b = beta - mean*s; pack [s_f;b_f;s_o;b_o]
        # as [8, 128] -> transpose -> [128, 8] whose col 2q+t is chunk t.
        # Emitted mid-stream (i==3) so the ACT/DVE waits on the slow SWDGE
        # param loads don't block the f1T eviction pipeline. ----
        setup = ctx.enter_context(tc.tile_pool(name="setup", bufs=1))
        bn_cols = setup.tile([128, 8], dt.float32, name="bn_cols")
        sf = bn_cols[:, 0:2]
        bf = bn_cols[:, 2:4]
        so = bn_cols[:, 4:6]
        bo = bn_cols[:, 6:8]

        def emit_bn_prep(ps_pool):
            # separate [2, 128] tiles (engine writes must start at partition 0)
            sb_rows = []
            for pre in ("f", "o"):
                s = setup.tile([2, 128], dt.float32, name=f"s_{pre}", tag=f"s_{pre}")
                b = setup.tile([2, 128], dt.float32, name=f"b_{pre}", tag=f"b_{pre}")
                tmp = setup.tile([2, 128], dt.float32, name=f"tmp_{pre}", tag=f"tmp_{pre}")
                nc.scalar.activation(
                    out=tmp, in_=bnt[f"bn_{pre}_var"], func=AF.Sqrt, bias=eps_t, scale=1.0
                )
                nc.vector.reciprocal(out=tmp, in_=tmp)
                nc.vector.tensor_mul(s, bnt[f"bn_{pre}_gamma"], tmp)
                nc.vector.tensor_mul(tmp, bnt[f"bn_{pre}_mean"], s)
                nc.vector.tensor_sub(b, bnt[f"bn_{pre}_beta"], tmp)
                sb_rows += [s, b]
            bn_ps_t = ps_pool.tile([128, 8], dt.float32, name="bn_ps_t", tag="bn_ps_t")
            for qi, rt in enumerate(sb_rows):
                nc.tensor.transpose(bn_ps_t[:, 2 * qi : 2 * qi + 2], rt, ident[0:2, 0:2])
            nc.vector.tensor_copy(bn_cols, bn_ps_t)

        alg = ctx.enter_context(tc.tile_pool(name="alg", bufs=1))

        def emit_rhs_g():
            # rhs = [diag(sf) Wf | bf] per c-chunk (feeds g_ext = mid @ rhs)
            for cc in range(2):
                r = alg.tile([128, C + 1], dt.float32, name=f"rhs_g{cc}", tag=f"rhs_g{cc}")
                nc.vector.tensor_scalar(
                    out=r[:, 0:C], in0=wf[cc], scalar1=sf[:, cc : cc + 1],
                    scalar2=None, op0=OP.mult,
                )
                nc.vector.tensor_copy(r[:, C : C + 1], bf[:, cc : cc + 1])
                rhs_g.append(r)

        # ident2: same values as ident, but its last column is rewritten with
        # a dependency on piece 3 (out = 0*piece + ident). The W transposes
        # consume ident2, which pins them late in the PE queue — the Tile
        # scheduler otherwise hoists them to the queue front, where their
        # wait on the slow SWDGE weight loads blocks the piece transposes.
        ident2 = const.tile([128, 128], dt.float32, name="ident2")

        def emit_ident2():
            nc.vector.tensor_copy(ident2, ident)
            nc.vector.scalar_tensor_tensor(
                out=ident2[:, 127:128], in0=F_t[0][3][:, 0:1].bitcast(dt.float32),
                scalar=0.0, in1=ident[:, 127:128], op0=OP.mult, op1=OP.add,
            )

        def emit_wT(w_ps_pool):
            # W2T[j][128, 256] and W1T[j][128, 256] via identity-matmul
            # transposes of W_out blocks (j = input-channel chunk); both share
            # one [128, 512] PSUM bank-tile per chunk.
            for jc in range(2):
                W_ps = w_ps_pool.tile([128, 2 * C], dt.float32, name="W_ps", tag="W_ps")
                W2T_ps = W_ps[:, 0:C]
                W1T_ps = W_ps[:, C : 2 * C]
                for oc in range(2):
                    nc.tensor.matmul(
                        W2T_ps[:, oc * 128 : (oc + 1) * 128],
                        lhsT=wo[oc][:, C + jc * 128 : C + (jc + 1) * 128],
                        rhs=ident2,
                        start=(oc == 0),
                        stop=(oc == 1),
                    )
                    nc.tensor.matmul(
                        W1T_ps[:, oc * 128 : (oc + 1) * 128],
                        lhsT=wo[oc][:, jc * 128 : (jc + 1) * 128],
                        rhs=ident2,
                        start=(oc == 0),
                        stop=(oc == 1),
                    )
                t = alg.tile([128, C], dt.float32, name=f"W2T{jc}", tag=f"W2T{jc}")
                nc.vector.tensor_copy(t, W2T_ps)
                W2T_sb.append(t)
                t1 = alg.tile([128, C], dt.float32, name=f"W1T{jc}", tag=f"W1T{jc}")
                nc.scalar.copy(t1, W1T_ps)
                W1T_sb.append(t1)

        rhs_g = []
        W2T_sb = []
        W1T_sb = []

        # ---- mid = fbu @ F^T via per-h PE transposes, accumulated in PSUM;
        # fbuT (exact binary masks) is the cheap stationary operand, the
        # transposed F blocks stream as the moving operand ----
        midT_sb = alg.tile([128, 6], dt.float32, name="midT_sb")
        with tc.tile_pool(name="midps", bufs=1, space="PSUM") as midps:
            mid_ps = midps.tile([3, C], dt.float32, name="mid_ps")
            with tc.tile_pool(name="tr_ps", bufs=4, space="PSUM") as tr_ps_pool, \
                 tc.tile_pool(name="f1T_pool", bufs=8) as f1T_pool, \
                 tc.tile_pool(name="w_ps_pool", bufs=1, space="PSUM") as w_ps_pool:
                GPP = 64 // NPIECE  # hp-groups per piece
                for i in range(NPIECE):
                    if i == 3:
                        emit_ident2()
                        emit_bn_prep(w_ps_pool)
                        emit_rhs_g()
                        emit_wT(w_ps_pool)
                    for g in range(GPP):
                        hp = GPP * i + g
                        tps = tr_ps_pool.tile([128, 512], F32R, name="tps")
                        for q in range(4):
                            h = 2 * hp + q // 2
                            cc = q % 2
                            nc.tensor.transpose(
                                tps[:, q * 128 : (q + 1) * 128],
                                f_slice(cc, h * 128, 128),
                                ident_r,
                            )
                        f1T = f1T_pool.tile([128, 512], F32R, name="f1T")
                        if hp % 2 == 0:
                            nc.vector.tensor_copy(f1T, tps)
                        else:
                            nc.scalar.copy(f1T, tps)
                        for q2 in range(2):
                            h = 2 * hp + q2
                            nc.tensor.matmul(
                                mid_ps[:, :],
                                lhsT=fbuT[:, h, :],
                                rhs=f1T[:, q2 * 256 : (q2 + 1) * 256],
                                start=(h == 0),
                                stop=(h == 127),
                            )
            mid_sb = alg.tile([3, C], dt.float32, name="mid_sb")
            nc.vector.tensor_copy(mid_sb, mid_ps)

        # mid^T via two small PE transposes (evicted in parallel on DVE/ACT)
        with tc.tile_pool(name="mt_ps", bufs=1, space="PSUM") as mt_ps_pool:
            mT_ps = mt_ps_pool.tile([128, 8], dt.float32, name="mT_ps")
            for cc in range(2):
                nc.tensor.transpose(
                    mT_ps[:, 4 * cc : 4 * cc + 3],
                    mid_sb[:, cc * 128 : (cc + 1) * 128],
                    ident[0:3, 0:3],
                )
            nc.vector.tensor_copy(midT_sb[:, 0:3], mT_ps[:, 0:3])
            nc.scalar.copy(midT_sb[:, 3:6], mT_ps[:, 4:7])

        # ---- tiny algebra: {gext, S} -> WeffT (+W1T during eviction), beff.
        # Dummy transposes between stages keep the PE p-state up while the
        # evictions round-trip through DVE/ACT. ----
        with tc.tile_pool(name="warm_psp", bufs=1, space="PSUM") as warm_ps_pool, \
             tc.tile_pool(name="alg_ps", bufs=1, space="PSUM") as alg_ps:
            warm_ps = warm_ps_pool.tile([128, 128], dt.float32, name="warm_ps")

            def keep_warm(src_ap, n):
                # cheap transposes whose input depends on a just-evicted tile,
                # so the Tile scheduler cannot hoist them early; they keep the
                # PE p-state up while the next stage's eviction round-trips
                for _ in range(n):
                    nc.tensor.transpose(warm_ps[:, 0:3], src_ap, ident[0:3, 0:3])

            keep_warm(mid_sb[0:3, 0:128], 3)
            # gext = mid @ [diag(sf) Wf | bf] : [3, C+1]; col C is u
            gext_ps = alg_ps.tile([3, C + 1], dt.float32, name="gext_ps", tag="gext_ps")
            for cc in range(2):
                nc.tensor.matmul(
                    gext_ps,
                    lhsT=midT_sb[:, cc * 3 : (cc + 1) * 3],
                    rhs=rhs_g[cc],
                    start=(cc == 0),
                    stop=(cc == 1),
                )
            # S = mid @ W2^T : [3, C]
            S_ps = alg_ps.tile([3, C], dt.float32, name="S_ps", tag="S_ps")
            for jc in range(2):
                nc.tensor.matmul(
                    S_ps,
                    lhsT=midT_sb[:, jc * 3 : (jc + 1) * 3],
                    rhs=W2T_sb[jc],
                    start=(jc == 0),
                    stop=(jc == 1),
                )
            gext_sb = alg.tile([3, C + 1], dt.float32, name="gext_sb")
            nc.vector.tensor_copy(gext_sb, gext_ps)
            S_sb = alg.tile([3, C], dt.float32, name="S_sb")
            nc.scalar.copy(S_sb, S_ps)
            keep_warm(gext_sb[0:3, 0:128], 4)

            # WeffT[cc] = W1T[cc] + g^T @ S (single K=3 matmul per c-chunk;
            # the W1T add happens during the PSUM eviction)
            WeffT_sb = []
            Wt_ps = []
            for cc in range(2):
                p = alg_ps.tile([128, C], dt.float32, name=f"Wt_ps{cc}", tag=f"Wt_ps{cc}")
                nc.tensor.matmul(
                    p, lhsT=gext_sb[:, cc * 128 : (cc + 1) * 128], rhs=S_sb,
                    start=True, stop=True,
                )
                Wt_ps.append(p)
            # beff = so * (S^T @ u) + bo  (off the first-final-matmul path)
            beff = alg.tile([128, 2], dt.float32, name="beff")
            wv_ps = []
            for oc in range(2):
                p = alg_ps.tile([128, 1], dt.float32, name=f"wv_ps{oc}", tag=f"wv_ps{oc}")
                nc.tensor.matmul(
                    p, lhsT=S_sb[:, oc * 128 : (oc + 1) * 128],
                    rhs=gext_sb[:, C : C + 1],
                    start=True, stop=True,
                )
                wv_ps.append(p)
            for cc in range(2):
                t = alg.tile([128, C], F32R, name=f"WeffT{cc}", tag=f"WeffT{cc}")
                nc.vector.tensor_tensor(t, Wt_ps[cc], W1T_sb[cc], OP.add)
                WeffT_sb.append(t)
            for oc in range(2):
                nc.vector.tensor_scalar(
                    out=beff[:, oc : oc + 1], in0=wv_ps[oc],
                    scalar1=so[:, oc : oc + 1], scalar2=bo[:, oc : oc + 1],
                    op0=OP.mult, op1=OP.add,
                )

        # ---- final: out = so * (Weff @ F) + beff, streamed over n ----
        # 2-bank PSUM super-tiles: 4 matmuls (2 n-halves x 2 c-chunks). Two
        # super-tiles pair into one [128, 2048] SBUF tile whose halves evict
        # on DVE and ACT in parallel, then one 1 MB store on the sync ring.
        NT = 512
        with tc.tile_pool(name="fin_ps", bufs=4, space="PSUM") as fin_ps, \
             tc.tile_pool(name="osb", bufs=4) as osb_pool:
            for oc in range(2):
                for gg in range(HW // (4 * NT)):
                    ot = osb_pool.tile([128, 4 * NT], dt.float32, name="ot")
                    for half in range(2):
                        ps2 = fin_ps.tile([128, 2 * NT], dt.float32, name="ps2")
                        for cc in range(2):
                            for t in range(2):
                                nt = 4 * gg + 2 * half + t
                                nc.tensor.matmul(
                                    ps2[:, t * NT : (t + 1) * NT],
                                    lhsT=WeffT_sb[cc][:, oc * 128 : (oc + 1) * 128],
                                    rhs=f_slice(cc, nt * NT, NT),
                                    start=(cc == 0),
                                    stop=(cc == 1),
                                )
                        dst = ot[:, half * 2 * NT : (half + 1) * 2 * NT]
                        if half == 0:
                            nc.vector.tensor_scalar(
                                out=dst, in0=ps2, scalar1=so[:, oc : oc + 1],
                                scalar2=beff[:, oc : oc + 1], op0=OP.mult, op1=OP.add,
                            )
                        else:
                            nc.scalar.activation(
                                out=dst, in_=ps2, func=AF.Identity,
                                bias=beff[:, oc : oc + 1], scale=so[:, oc : oc + 1],
                            )
                    nc.sync.dma_start(
                        out=out_d[
                            oc * 128 : (oc + 1) * 128, 4 * gg * NT : (4 * gg + 4) * NT
                        ],
                        in_=ot,
                    )

    _split_drain_waits(nc)
    return nc


_NC_CACHE = None


def _get_nc():
    global _NC_CACHE
    if _NC_CACHE is None:
        _NC_CACHE = build_nc()
    return _NC_CACHE


def kernel(**inputs):
    feature = np.asarray(inputs["feature"], dtype=np.float32)
    m = np.asarray(inputs["m"], dtype=np.float32)
    shared = {}
    shared["w_feat"] = np.asarray(inputs["w_feat"], dtype=np.float32)
    shared["w_out"] = np.asarray(inputs["w_out"], dtype=np.float32)
    for pre in ("f", "o"):
        for nm in ("gamma", "beta", "mean", "var"):
            key = f"bn_{pre}_{nm}"
            shared[key] = np.asarray(inputs[key], dtype=np.float32)

    nc = _get_nc()
    in_maps = []
    for i in range(NCORES):
        im = dict(shared)
        im["feature"] = np.ascontiguousarray(feature[i].reshape(C, HW))
        im["m"] = np.ascontiguousarray(m[i].reshape(H, W))
        in_maps.append(im)

    res = run_bass_kernel_spmd(nc, in_maps, core_ids=list(range(NCORES)))
    out = np.stack([res.results[i]["out"].reshape(C, H, W) for i in range(NCORES)])
    return out


# revision 22
# speedup vs baseline: 1.1391x; 1.1391x over previous
"""Trainium2 Bass kernel for nn_BDFM_46428596469849.

Per-batch math (B=8, C=256, H=W=128, HW=16384):
    m   = relu(m); z = (m > 0.3)
    er  = minpool4x4(z, SAME, border=1); di = maxpool4x4(z, SAME, border=0)
    fbu = [er, 1-di, di-er]                          # [3, HW]
    mid = fbu @ F^T                                  # [3, C]
    cf  = bn_f(Wf @ F);  mid1 = mid @ cf;  mid2 = mid^T @ mid1
    out = bn_o(W_out @ [F; mid2])

The chain collapses algebraically: with sf/bf (resp. so/bo) the BN scale/bias,
    g    = mid @ [diag(sf) Wf | bf]       # [3, C+1] (col C is u = mid @ bf)
    S    = mid @ W2^T                     # [3, C]   (W_out = [W1 | W2])
    WeffT= W1^T + g^T @ S                 # [C, C]   (= (W1 + W2 @ mid^T g)^T)
    wv   = S^T @ u                        # [C]      (= W2 @ mid^T @ u)
    out  = diag(so) @ WeffT^T @ F + (so*wv + bo) 1^T
so each batch element needs: the midT reduction (one pass over F with PE
transposes), a three-stage tiny-algebra chain, and one C x C x HW matmul
streamed over resident F.

Sharding: data-parallel, one batch element per NeuronCore (8 cores).
"""

import os
import sys

for _p in ("/opt/trn_rl_repo", "/root/.axon_site/_ro/trn_rl_repo"):
    if os.path.isdir(_p) and _p not in sys.path:
        sys.path.insert(0, _p)

import numpy as np

import concourse.bass as bass
import concourse.mybir as mybir
import concourse.tile as tile
from concourse.bass_utils import run_bass_kernel_spmd
from concourse.masks import make_identity

dt = mybir.dt
AF = mybir.ActivationFunctionType
OP = mybir.AluOpType

B, C, H, W = 8, 256, 128, 128
HW = H * W
NCORES = 8
EPS = 1e-5
F32R = dt.float32r


def _split_drain_waits(nc, max_waits=1):
    # Walrus codegen rejects instructions carrying more than a couple of
    # semaphore waits (CTRL drains and DMA descriptors in particular). Hoist
    # excess waits onto preceding NoOps on the same engine queue — the queue
    # executes in order, so the waits are satisfied before the instruction.
    for f in nc.m.functions:
        for bb in f.blocks:
            new_insts = []
            for inst in bb.instructions:
                si = inst.sync_info
                if si is not None and si.on_wait and len(si.on_wait) > max_waits:
                    waits = list(si.on_wait)
                    while len(waits) > max_waits:
                        chunk, waits = waits[:max_waits], waits[max_waits:]
                        pre = mybir.InstNoOp(
                            name=f"I-wsplit-{nc.next_id()}",
                            engine=inst.engine,
                            sync_info=mybir.SyncInfo(on_wait=chunk, on_update=[]),
                        )
                        nc.inst_map[pre.name] = pre
                        new_insts.append(pre)
                    inst.sync_info = mybir.SyncInfo(
                        on_wait=waits, on_update=list(si.on_update)
                    )
                new_insts.append(inst)
            bb.instructions[:] = new_insts


def build_nc():
    from contextlib import ExitStack

    nc = bass.Bass("TRN2", target_bir_lowering=False)

    feat = nc.declare_dram_parameter("feature", [C, HW], dt.float32, isOutput=False)
    m_in = nc.declare_dram_parameter("m", [H, W], dt.float32, isOutput=False)
    wfeat = nc.declare_dram_parameter("w_feat", [C, C], dt.float32, isOutput=False)
    wout = nc.declare_dram_parameter("w_out", [C, 2 * C], dt.float32, isOutput=False)
    bnp = {}
    for pre in ("f", "o"):
        for nm in ("gamma", "beta", "mean", "var"):
            key = f"bn_{pre}_{nm}"
            bnp[key] = nc.declare_dram_parameter(key, [C], dt.float32, isOutput=False)
    out_d = nc.declare_dram_parameter("out", [C, HW], dt.float32, isOutput=True)

    with tile.TileContext(nc) as tc, ExitStack() as ctx:
        const = ctx.enter_context(tc.tile_pool(name="const", bufs=1))
        ident = const.tile([128, 128], dt.float32, name="ident")
        make_identity(nc, ident)
        ident_r = const.tile([128, 128], F32R, name="ident_r")
        nc.vector.tensor_copy(ident_r, ident)
        eps_t = const.tile([2, 1], dt.float32, name="eps_t")
        nc.vector.memset(eps_t, EPS)

        # ---- resident feature in float32r, as per-piece tiles ----
        # fp32 DRAM bits land directly as f32r via bitcast; separate per-piece
        # tiles keep the dependency granularity fine so the midT phase
        # overlaps the load. 2048-col pieces = 1 MB / 8 KB-per-partition DMAs.
        NPIECE = 8
        PIECE = HW // NPIECE
        fpool = ctx.enter_context(tc.tile_pool(name="fpool", bufs=1))
        F_t = [
            [
                fpool.tile([128, PIECE], F32R, name=f"F{cc}_{i}", tag=f"F{cc}_{i}")
                for i in range(NPIECE)
            ]
            for cc in range(2)
        ]

        def f_slice(cc, col0, width):
            i = col0 // PIECE
            off = col0 % PIECE
            assert off + width <= PIECE
            return F_t[cc][i][:, off : off + width]

        # ---- ACT activation-table preload (Sqrt), off the critical path ----
        act_warm = const.tile([2, 1], dt.float32, name="act_warm")
        nc.scalar.activation(out=act_warm, in_=eps_t, func=AF.Sqrt, bias=eps_t, scale=1.0)

        # ---- m first on the sync ring (morphology is on the critical path) --
        m_sb = const.tile([128, 128], dt.float32, name="m_sb")
        nc.sync.dma_start(out=m_sb, in_=m_in[:, :])

        # ---- feature loads: all on the sync HWDGE ring, in piece order. The
        # scalar(ACT) ring carries NO loads — the HWDGE ring depth (~2) would
        # block the ACT sequencer, which must stay free for the f1T
        # evictions; SWDGE loads measured ~25% slower than HWDGE here. ----
        for i in range(NPIECE):
            for cc in range(2):
                nc.sync.dma_start(
                    out=F_t[cc][i][:],
                    in_=feat[
                        cc * 128 : (cc + 1) * 128, i * PIECE : (i + 1) * PIECE
                    ].bitcast(F32R),
                )

        # ---- small inputs, behind the odd F pieces on the gpsimd queue ----
        wf = []
        wo = []
        for oc in range(2):
            t = const.tile([128, C], dt.float32, name=f"wf{oc}", tag=f"wf{oc}")
            nc.gpsimd.dma_start(out=t, in_=wfeat[oc * 128 : (oc + 1) * 128, :])
            wf.append(t)
            t2 = const.tile([128, 2 * C], dt.float32, name=f"wo{oc}", tag=f"wo{oc}")
            nc.gpsimd.dma_start(out=t2, in_=wout[oc * 128 : (oc + 1) * 128, :])
            wo.append(t2)

        # BN params as [2, 128] (2 contiguous 512B descriptors each, instead
        # of 256 4-byte gather descriptors in [128, 2] layout).
        bnt = {}
        for key, hdl in bnp.items():
            t = const.tile([2, 128], dt.float32, name=f"t_{key}", tag=f"t_{key}")
            nc.gpsimd.dma_start(out=t, in_=hdl[:].rearrange("(t p) -> t p", p=128))
            bnt[key] = t

        # ---- morphology: separable 4x4 window (offsets -1..+2), both passes
        # along the free dim with a PE transpose in between; border = the
        # reduction identity (matches reduce_window SAME + init value) ----
        mor = ctx.enter_context(tc.tile_pool(name="mor", bufs=1))

        def pool1d_free(eng, src, op, border, label):
            padd = mor.tile([128, 131], dt.float32, name=f"pad_{label}", tag=f"pad_{label}")
            eng.memset(padd, border)
            eng.tensor_copy(padd[:, 1:129], src)
            a = mor.tile([128, 130], dt.float32, name=f"a_{label}", tag=f"a_{label}")
            eng.tensor_tensor(a, padd[:, 0:130], padd[:, 1:131], op)
            r = mor.tile([128, 128], dt.float32, name=f"r_{label}", tag=f"r_{label}")
            eng.tensor_tensor(r, a[:, 0:128], a[:, 2:130], op)
            return r

        z = mor.tile([128, 128], dt.float32, name="z")
        nc.vector.tensor_scalar(out=z, in0=m_sb, scalar1=0.3, scalar2=None, op0=OP.is_gt)
        erw = pool1d_free(nc.vector, z, OP.min, 1.0, "er1")  # [h, w] pooled over w
        diw = pool1d_free(nc.vector, z, OP.max, 0.0, "di1")
        with tc.tile_pool(name="mor_ps", bufs=1, space="PSUM") as mor_ps:
            er_ps = mor_ps.tile([128, 128], dt.float32, name="er_ps", tag="er_ps")
            nc.tensor.transpose(er_ps, erw, ident)
            erwT = mor.tile([128, 128], dt.float32, name="erwT")
            nc.vector.tensor_copy(erwT, er_ps)
            di_ps = mor_ps.tile([128, 128], dt.float32, name="di_ps", tag="di_ps")
            nc.tensor.transpose(di_ps, diw, ident)
            diwT = mor.tile([128, 128], dt.float32, name="diwT")
            nc.vector.tensor_copy(diwT, di_ps)
        erT = pool1d_free(nc.vector, erwT, OP.min, 1.0, "er2")  # [w, h] pooled over h
        diT = pool1d_free(nc.vector, diwT, OP.max, 0.0, "di2")

        # fbuT[w, h, k] = fbu[k, h*128+w]
        fbuT = mor.tile([128, 128, 3], F32R, name="fbuT")
        nc.vector.tensor_copy(fbuT[:, :, 0], erT)
        nc.vector.tensor_scalar(
            out=fbuT[:, :, 1], in0=diT, scalar1=-1.0, scalar2=1.0, op0=OP.mult, op1=OP.add
        )
        nc.vector.tensor_tensor(fbuT[:, :, 2], diT, erT, OP.subtract)

        # ---- BN scale/bias in [2, 128] layout, then one packed PE transpose:
        # s = gamma*rsqrt(var+eps), b = beta - mean*s; pack [s_f;b_f;s_o;b_o]
        # as [8, 128] -> transpose -> [128, 8] whose col 2q+t is chunk t.
        # Emitted mid-stream (i==3) so the ACT/DVE waits on the slow SWDGE
        # param loads don't block the f1T eviction pipeline. ----
        setup = ctx.enter_context(tc.tile_pool(name="setup", bufs=1))
        bn_cols = setup.tile([128, 8], dt.float32, name="bn_cols")
        sf = bn_cols[:, 0:2]
        bf = bn_cols[:, 2:4]
        so = bn_cols[:, 4:6]
        bo = bn_cols[:, 6:8]

        def emit_bn_prep(ps_pool):
            # separate [2, 128] tiles (engine writes must start at partition 0)
            sb_rows = []
            for pre in ("f", "o"):
                s = setup.tile([2, 128], dt.float32, name=f"s_{pre}", tag=f"s_{pre}")
                b = setup.tile([2, 128], dt.float32, name=f"b_{pre}", tag=f"b_{pre}")
                tmp = setup.tile([2, 128], dt.float32, name=f"tmp_{pre}", tag=f"tmp_{pre}")
                nc.scalar.activation(
                    out=tmp, in_=bnt[f"bn_{pre}_var"], func=AF.Sqrt, bias=eps_t, scale=1.0
                )
                nc.vector.reciprocal(out=tmp, in_=tmp)
                nc.vector.tensor_mul(s, bnt[f"bn_{pre}_gamma"], tmp)
                nc.vector.tensor_mul(tmp, bnt[f"bn_{pre}_mean"], s)
                nc.vector.tensor_sub(b, bnt[f"bn_{pre}_beta"], tmp)
                sb_rows += [s, b]
            bn_ps_t = ps_pool.tile([128, 8], dt.float32, name="bn_ps_t", tag="bn_ps_t")
            for qi, rt in enumerate(sb_rows):
                nc.tensor.transpose(bn_ps_t[:, 2 * qi : 2 * qi + 2], rt, ident[0:2, 0:2])
            nc.vector.tensor_copy(bn_cols, bn_ps_t)

        alg = ctx.enter_context(tc.tile_pool(name="alg", bufs=1))

        def emit_rhs_g():
            # rhs = [diag(sf) Wf | bf] per c-chunk (feeds g_ext = mid @ rhs)
            for cc in range(2):
                r = alg.tile([128, C + 1], dt.float32, name=f"rhs_g{cc}", tag=f"rhs_g{cc}")
                nc.vector.tensor_scalar(
                    out=r[:, 0:C], in0=wf[cc], scalar1=sf[:, cc : cc + 1],
                    scalar2=None, op0=OP.mult,
                )
                nc.vector.tensor_copy(r[:, C : C + 1], bf[:, cc : cc + 1])
                rhs_g.append(r)

        # ident2: same values as ident, but its last column is rewritten with
        # a dependency on piece 3 (out = 0*piece + ident). The W transposes
        # consume ident2, which pins them late in the PE queue — the Tile
        # scheduler otherwise hoists them to the queue front, where their
        # wait on the slow SWDGE weight loads blocks the piece transposes.
        ident2 = const.tile([128, 128], dt.float32, name="ident2")

        def emit_ident2():
            nc.vector.tensor_copy(ident2, ident)
            nc.vector.scalar_tensor_tensor(
                out=ident2[:, 127:128], in0=F_t[0][3][:, 0:1].bitcast(dt.float32),
                scalar=0.0, in1=ident[:, 127:128], op0=OP.mult, op1=OP.add,
            )

        def emit_wT(w_ps_pool):
            # W2T[j][128, 256] and W1T[j][128, 256] via identity-matmul
            # transposes of W_out blocks (j = input-channel chunk); both share
            # one [128, 512] PSUM bank-tile per chunk.
            for jc in range(2):
                W_ps = w_ps_pool.tile([128, 2 * C], dt.float32, name="W_ps", tag="W_ps")
                W2T_ps = W_ps[:, 0:C]
                W1T_ps = W_ps[:, C : 2 * C]
                for oc in range(2):
                    nc.tensor.matmul(
                        W2T_ps[:, oc * 128 : (oc + 1) * 128],
                        lhsT=wo[oc][:, C + jc * 128 : C + (jc + 1) * 128],
                        rhs=ident2,
                        start=(oc == 0),
                        stop=(oc == 1),
                    )
                    nc.tensor.matmul(
                        W1T_ps[:, oc * 128 : (oc + 1) * 128],
                        lhsT=wo[oc][:, jc * 128 : (jc + 1) * 128],
                        rhs=ident2,
                        start=(oc == 0),
                        stop=(oc == 1),
                    )
                t = alg.tile([128, C], dt.float32, name=f"W2T{jc}", tag=f"W2T{jc}")
                nc.vector.tensor_copy(t, W2T_ps)
                W2T_sb.append(t)
                t1 = alg.tile([128, C], dt.float32, name=f"W1T{jc}", tag=f"W1T{jc}")
                nc.scalar.copy(t1, W1T_ps)
                W1T_sb.append(t1)

        rhs_g = []
        W2T_sb = []
        W1T_sb = []

        # ---- mid = fbu @ F^T via per-h PE transposes, accumulated in PSUM;
        # fbuT (exact binary masks) is the cheap stationary operand, the
        # transposed F blocks stream as the moving operand ----
        midT_sb = alg.tile([128, 6], dt.float32, name="midT_sb")
        with tc.tile_pool(name="midps", bufs=1, space="PSUM") as midps:
            mid_ps = midps.tile([3, C], dt.float32, name="mid_ps")
            with tc.tile_pool(name="tr_ps", bufs=4, space="PSUM") as tr_ps_pool, \
                 tc.tile_pool(name="f1T_pool", bufs=8) as f1T_pool, \
                 tc.tile_pool(name="w_ps_pool", bufs=1, space="PSUM") as w_ps_pool:
                GPP = 64 // NPIECE  # hp-groups per piece
                for i in range(NPIECE):
                    if i == 3:
                        emit_ident2()
                        emit_bn_prep(w_ps_pool)
                        emit_rhs_g()
                        emit_wT(w_ps_pool)
                    for g in range(GPP):
                        hp = GPP * i + g
                        tps = tr_ps_pool.tile([128, 512], F32R, name="tps")
                        for q in range(4):
                            h = 2 * hp + q // 2
                            cc = q % 2
                            nc.tensor.transpose(
                                tps[:, q * 128 : (q + 1) * 128],
                                f_slice(cc, h * 128, 128),
                                ident_r,
                            )
                        f1T = f1T_pool.tile([128, 512], F32R, name="f1T")
                        if hp % 2 == 0:
                            nc.vector.tensor_copy(f1T, tps)
                        else:
                            nc.scalar.copy(f1T, tps)
                        for q2 in range(2):
                            h = 2 * hp + q2
                            nc.tensor.matmul(
                                mid_ps[:, :],
                                lhsT=fbuT[:, h, :],
                                rhs=f1T[:, q2 * 256 : (q2 + 1) * 256],
                                start=(h == 0),
                                stop=(h == 127),
                            )
            mid_sb = alg.tile([3, C], dt.float32, name="mid_sb")
            nc.vector.tensor_copy(mid_sb, mid_ps)

        # mid^T via two small PE transposes (evicted in parallel on DVE/ACT)
        with tc.tile_pool(name="mt_ps", bufs=1, space="PSUM") as mt_ps_pool:
            mT_ps = mt_ps_pool.tile([128, 8], dt.float32, name="mT_ps")
            for cc in range(2):
                nc.tensor.transpose(
                    mT_ps[:, 4 * cc : 4 * cc + 3],
                    mid_sb[:, cc * 128 : (cc + 1) * 128],
                    ident[0:3, 0:3],
                )
            nc.vector.tensor_copy(midT_sb[:, 0:3], mT_ps[:, 0:3])
            nc.scalar.copy(midT_sb[:, 3:6], mT_ps[:, 4:7])

        # ---- tiny algebra: {gext, S} -> WeffT (+W1T during eviction), beff.
        # Dummy transposes between stages keep the PE p-state up while the
        # evictions round-trip through DVE/ACT. ----
        with tc.tile_pool(name="warm_psp", bufs=1, space="PSUM") as warm_ps_pool, \
             tc.tile_pool(name="alg_ps", bufs=1, space="PSUM") as alg_ps:
            warm_ps = warm_ps_pool.tile([128, 128], dt.float32, name="warm_ps")

            def keep_warm(src_ap, n):
                # cheap transposes whose input depends on a just-evicted tile,
                # so the Tile scheduler cannot hoist them early; they keep the
                # PE p-state up while the next stage's eviction round-trips
                for _ in range(n):
                    nc.tensor.transpose(warm_ps[:, 0:3], src_ap, ident[0:3, 0:3])

            keep_warm(mid_sb[0:3, 0:128], 3)
            # gext = mid @ [diag(sf) Wf | bf] : [3, C+1]; col C is u
            gext_ps = alg_ps.tile([3, C + 1], dt.float32, name="gext_ps", tag="gext_ps")
            for cc in range(2):
                nc.tensor.matmul(
                    gext_ps,
                    lhsT=midT_sb[:, cc * 3 : (cc + 1) * 3],
                    rhs=rhs_g[cc],
                    start=(cc == 0),
                    stop=(cc == 1),
                )
            # S = mid @ W2^T : [3, C]
            S_ps = alg_ps.tile([3, C], dt.float32, name="S_ps", tag="S_ps")
            for jc in range(2):
                nc.tensor.matmul(
                    S_ps,
                    lhsT=midT_sb[:, jc * 3 : (jc + 1) * 3],
                    rhs=W2T_sb[jc],
                    start=(jc == 0),
                    stop=(jc == 1),
                )
            gext_sb = alg.tile([3, C + 1], dt.float32, name="gext_sb")
            nc.vector.tensor_copy(gext_sb, gext_ps)
            S_sb = alg.tile([3, C], dt.float32, name="S_sb")
            nc.scalar.copy(S_sb, S_ps)
            keep_warm(gext_sb[0:3, 0:128], 4)

            # WeffT[cc] = W1T[cc] + g^T @ S (single K=3 matmul per c-chunk;
            # the W1T add happens during the PSUM eviction)
            WeffT_sb = []
            Wt_ps = []
            for cc in range(2):
                p = alg_ps.tile([128, C], dt.float32, name=f"Wt_ps{cc}", tag=f"Wt_ps{cc}")
                nc.tensor.matmul(
                    p, lhsT=gext_sb[:, cc * 128 : (cc + 1) * 128], rhs=S_sb,
                    start=True, stop=True,
                )
                Wt_ps.append(p)
            # beff = so * (S^T @ u) + bo  (off the first-final-matmul path)
            beff = alg.tile([128, 2], dt.float32, name="beff")
            wv_ps = []
            for oc in range(2):
                p = alg_ps.tile([128, 1], dt.float32, name=f"wv_ps{oc}", tag=f"wv_ps{oc}")
                nc.tensor.matmul(
                    p, lhsT=S_sb[:, oc * 128 : (oc + 1) * 128],
                    rhs=gext_sb[:, C : C + 1],
                    start=True, stop=True,
                )
                wv_ps.append(p)
            for cc in range(2):
                t = alg.tile([128, C], F32R, name=f"WeffT{cc}", tag=f"WeffT{cc}")
                nc.vector.tensor_tensor(t, Wt_ps[cc], W1T_sb[cc], OP.add)
                WeffT_sb.append(t)
            for oc in range(2):
                nc.vector.tensor_scalar(
                    out=beff[:, oc : oc + 1], in0=wv_ps[oc],
                    scalar1=so[:, oc : oc + 1], scalar2=bo[:, oc : oc + 1],
                    op0=OP.mult, op1=OP.add,
                )

        # ---- final: out = so * (Weff @ F) + beff, streamed over n ----
        # 2-bank PSUM super-tiles: 4 matmuls (2 n-halves x 2 c-chunks). Two
        # super-tiles pair into one [128, 2048] SBUF tile whose halves evict
        # on DVE and ACT in parallel, then one 1 MB store on the sync ring.
        NT = 512
        with tc.tile_pool(name="fin_ps", bufs=4, space="PSUM") as fin_ps, \
             tc.tile_pool(name="osb", bufs=4) as osb_pool:
            for oc in range(2):
                for gg in range(HW // (4 * NT)):
                    ot = osb_pool.tile([128, 4 * NT], dt.float32, name="ot")
                    for half in range(2):
                        ps2 = fin_ps.tile([128, 2 * NT], dt.float32, name="ps2")
                        for cc in range(2):
                            for t in range(2):
                                nt = 4 * gg + 2 * half + t
                                nc.tensor.matmul(
                                    ps2[:, t * NT : (t + 1) * NT],
                                    lhsT=WeffT_sb[cc][:, oc * 128 : (oc + 1) * 128],
                                    rhs=f_slice(cc, nt * NT, NT),
                                    start=(cc == 0),
                                    stop=(cc == 1),
                                )
                        dst = ot[:, half * 2 * NT : (half + 1) * 2 * NT]
                        if half == 0:
                            nc.vector.tensor_scalar(
                                out=dst, in0=ps2, scalar1=so[:, oc : oc + 1],
                                scalar2=beff[:, oc : oc + 1], op0=OP.mult, op1=OP.add,
                            )
                        else:
                            nc.scalar.activation(
                                out=dst, in_=ps2, func=AF.Identity,
                                bias=beff[:, oc : oc + 1], scale=so[:, oc : oc + 1],
                            )
                    nc.sync.dma_start(
                        out=out_d[
                            oc * 128 : (oc + 1) * 128, 4 * gg * NT : (4 * gg + 4) * NT
                        ],
                        in_=ot,
                    )

    _split_drain_waits(nc)
    return nc


_NC_CACHE = None


def _get_nc():
    global _NC_CACHE
    if _NC_CACHE is None:
        _NC_CACHE = build_nc()
    return _NC_CACHE


def kernel(**inputs):
    feature = np.asarray(inputs["feature"], dtype=np.float32)
    m = np.asarray(inputs["m"], dtype=np.float32)
    shared = {}
    shared["w_feat"] = np.asarray(inputs["w_feat"], dtype=np.float32)
    shared["w_out"] = np.asarray(inputs["w_out"], dtype=np.float32)
    for pre in ("f", "o"):
        for nm in ("gamma", "beta", "mean", "var"):
            key = f"bn_{pre}_{nm}"
            shared[key] = np.asarray(inputs[key], dtype=np.float32)

    nc = _get_nc()
    in_maps = []
    for i in range(NCORES):
        im = dict(shared)
        im["feature"] = np.ascontiguousarray(feature[i].reshape(C, HW))
        im["m"] = np.ascontiguousarray(m[i].reshape(H, W))
        in_maps.append(im)

    res = run_bass_kernel_spmd(nc, in_maps, core_ids=list(range(NCORES)))
    out = np.stack([res.results[i]["out"].reshape(C, H, W) for i in range(NCORES)])
    return out


# revision 25
# speedup vs baseline: 1.1995x; 1.0531x over previous
"""Trainium2 Bass kernel for nn_BDFM_46428596469849.

Per-batch math (B=8, C=256, H=W=128, HW=16384):
    m   = relu(m); z = (m > 0.3)
    er  = minpool4x4(z, SAME, border=1); di = maxpool4x4(z, SAME, border=0)
    fbu = [er, 1-di, di-er]                          # [3, HW]
    mid = fbu @ F^T                                  # [3, C]
    cf  = bn_f(Wf @ F);  mid1 = mid @ cf;  mid2 = mid^T @ mid1
    out = bn_o(W_out @ [F; mid2])

The chain collapses algebraically: with sf/bf (resp. so/bo) the BN scale/bias,
    g    = mid @ [diag(sf) Wf | bf]       # [3, C+1] (col C is u = mid @ bf)
    S    = mid @ W2^T                     # [3, C]   (W_out = [W1 | W2])
    WeffT= W1^T + g^T @ S                 # [C, C]   (= (W1 + W2 @ mid^T g)^T)
    wv   = S^T @ u                        # [C]      (= W2 @ mid^T @ u)
    out  = diag(so) @ WeffT^T @ F + (so*wv + bo) 1^T
so each batch element needs: the midT reduction (one pass over F with PE
transposes), a three-stage tiny-algebra chain, and one C x C x HW matmul
streamed over resident F.

Sharding: data-parallel, one batch element per NeuronCore (8 cores).
"""

import os
import sys

for _p in ("/opt/trn_rl_repo", "/root/.axon_site/_ro/trn_rl_repo"):
    if os.path.isdir(_p) and _p not in sys.path:
        sys.path.insert(0, _p)

import numpy as np

import concourse.bass as bass
import concourse.mybir as mybir
import concourse.tile as tile
from concourse.bass_utils import run_bass_kernel_spmd
from concourse.masks import make_identity

dt = mybir.dt
AF = mybir.ActivationFunctionType
OP = mybir.AluOpType

B, C, H, W = 8, 256, 128, 128
HW = H * W
NCORES = 8
EPS = 1e-5
F32R = dt.float32r


def _split_drain_waits(nc, max_waits=1):
    # Walrus codegen rejects instructions carrying more than a couple of
    # semaphore waits (CTRL drains and DMA descriptors in particular). Hoist
    # excess waits onto preceding NoOps on the same engine queue — the queue
    # executes in order, so the waits are satisfied before the instruction.
    for f in nc.m.functions:
        for bb in f.blocks:
            new_insts = []
            for inst in bb.instructions:
                si = inst.sync_info
                if si is not None and si.on_wait and len(si.on_wait) > max_waits:
                    waits = list(si.on_wait)
                    while len(waits) > max_waits:
                        chunk, waits = waits[:max_waits], waits[max_waits:]
                        pre = mybir.InstNoOp(
                            name=f"I-wsplit-{nc.next_id()}",
                            engine=inst.engine,
                            sync_info=mybir.SyncInfo(on_wait=chunk, on_update=[]),
                        )
                        nc.inst_map[pre.name] = pre
                        new_insts.append(pre)
                    inst.sync_info = mybir.SyncInfo(
                        on_wait=waits, on_update=list(si.on_update)
                    )
                new_insts.append(inst)
            bb.instructions[:] = new_insts


def build_nc():
    from contextlib import ExitStack

    nc = bass.Bass("TRN2", target_bir_lowering=False)

    feat = nc.declare_dram_parameter("feature", [C, HW], dt.float32, isOutput=False)
    m_in = nc.declare_dram_parameter("m", [H, W], dt.float32, isOutput=False)
    wfeat = nc.declare_dram_parameter("w_feat", [C, C], dt.float32, isOutput=False)
    wout = nc.declare_dram_parameter("w_out", [C, 2 * C], dt.float32, isOutput=False)
    bnp = {}
    for pre in ("f", "o"):
        for nm in ("gamma", "beta", "mean", "var"):
            key = f"bn_{pre}_{nm}"
            bnp[key] = nc.declare_dram_parameter(key, [C], dt.float32, isOutput=False)
    out_d = nc.declare_dram_parameter("out", [C, HW], dt.float32, isOutput=True)

    with tile.TileContext(nc) as tc, ExitStack() as ctx:
        const = ctx.enter_context(tc.tile_pool(name="const", bufs=1))
        ident = const.tile([128, 128], dt.float32, name="ident")
        make_identity(nc, ident)
        ident_r = const.tile([128, 128], F32R, name="ident_r")
        nc.vector.tensor_copy(ident_r, ident)
        eps_t = const.tile([2, 1], dt.float32, name="eps_t")
        nc.vector.memset(eps_t, EPS)

        # ---- resident feature in float32r, as per-piece tiles ----
        # fp32 DRAM bits land directly as f32r via bitcast; separate per-piece
        # tiles keep the dependency granularity fine so the midT phase
        # overlaps the load. 2048-col pieces = 1 MB / 8 KB-per-partition DMAs.
        NPIECE = 8
        PIECE = HW // NPIECE
        fpool = ctx.enter_context(tc.tile_pool(name="fpool", bufs=1))
        F_t = [
            [
                fpool.tile([128, PIECE], F32R, name=f"F{cc}_{i}", tag=f"F{cc}_{i}")
                for i in range(NPIECE)
            ]
            for cc in range(2)
        ]

        def f_slice(cc, col0, width):
            i = col0 // PIECE
            off = col0 % PIECE
            assert off + width <= PIECE
            return F_t[cc][i][:, off : off + width]

        # ---- ACT activation-table preload (Sqrt), off the critical path ----
        act_warm = const.tile([2, 1], dt.float32, name="act_warm")
        nc.scalar.activation(out=act_warm, in_=eps_t, func=AF.Sqrt, bias=eps_t, scale=1.0)

        # ---- m first on the sync ring (morphology is on the critical path) --
        m_sb = const.tile([128, 128], dt.float32, name="m_sb")
        nc.sync.dma_start(out=m_sb, in_=m_in[:, :])

        # ---- feature loads: all on the sync HWDGE ring, in piece order. The
        # scalar(ACT) ring carries NO loads — the HWDGE ring depth (~2) would
        # block the ACT sequencer, which must stay free for the f1T
        # evictions; SWDGE loads measured ~25% slower than HWDGE here. ----
        for i in range(NPIECE):
            for cc in range(2):
                nc.sync.dma_start(
                    out=F_t[cc][i][:],
                    in_=feat[
                        cc * 128 : (cc + 1) * 128, i * PIECE : (i + 1) * PIECE
                    ].bitcast(F32R),
                )

        # ---- morphology: separable 4x4 window (offsets -1..+2), both passes
        # along the free dim with a PE transpose in between; border = the
        # reduction identity (matches reduce_window SAME + init value) ----
        mor = ctx.enter_context(tc.tile_pool(name="mor", bufs=1))

        def pool1d_free(eng, src, op, border, label):
            padd = mor.tile([128, 131], dt.float32, name=f"pad_{label}", tag=f"pad_{label}")
            eng.memset(padd, border)
            eng.tensor_copy(padd[:, 1:129], src)
            a = mor.tile([128, 130], dt.float32, name=f"a_{label}", tag=f"a_{label}")
            eng.tensor_tensor(a, padd[:, 0:130], padd[:, 1:131], op)
            r = mor.tile([128, 128], dt.float32, name=f"r_{label}", tag=f"r_{label}")
            eng.tensor_tensor(r, a[:, 0:128], a[:, 2:130], op)
            return r

        hp_ctx = tc.high_priority()
        hp_ctx.__enter__()
        z = mor.tile([128, 128], dt.float32, name="z")
        nc.vector.tensor_scalar(out=z, in0=m_sb, scalar1=0.3, scalar2=None, op0=OP.is_gt)
        erw = pool1d_free(nc.vector, z, OP.min, 1.0, "er1")  # [h, w] pooled over w
        diw = pool1d_free(nc.vector, z, OP.max, 0.0, "di1")
        # er/di PSUM evictions go on ACT and the second pooling + fbuT build
        # on the (otherwise idle) GpSimd engine, so the DVE/ACT queues stay
        # clear for the f1T eviction pipeline that the mid matmuls depend on.
        with tc.tile_pool(name="mor_ps", bufs=1, space="PSUM") as mor_ps:
            er_ps = mor_ps.tile([128, 128], dt.float32, name="er_ps", tag="er_ps")
            nc.tensor.transpose(er_ps, erw, ident)
            erwT = mor.tile([128, 128], dt.float32, name="erwT")
            nc.scalar.copy(erwT, er_ps)
            di_ps = mor_ps.tile([128, 128], dt.float32, name="di_ps", tag="di_ps")
            nc.tensor.transpose(di_ps, diw, ident)
            diwT = mor.tile([128, 128], dt.float32, name="diwT")
            nc.scalar.copy(diwT, di_ps)
        erT = pool1d_free(nc.vector, erwT, OP.min, 1.0, "er2")  # [w, h] pooled over h
        diT = pool1d_free(nc.vector, diwT, OP.max, 0.0, "di2")

        # fbuT[w, h, k] = fbu[k, h*128+w]
        fbuT = mor.tile([128, 128, 3], F32R, name="fbuT")
        nc.vector.tensor_copy(fbuT[:, :, 0], erT)
        nc.vector.tensor_scalar(
            out=fbuT[:, :, 1], in0=diT, scalar1=-1.0, scalar2=1.0, op0=OP.mult, op1=OP.add
        )
        nc.vector.tensor_tensor(fbuT[:, :, 2], diT, erT, OP.subtract)
        hp_ctx.__exit__(None, None, None)

        # ---- small inputs, behind the odd F pieces on the gpsimd queue ----
        wf = []
        wo = []
        for oc in range(2):
            t = const.tile([128, C], dt.float32, name=f"wf{oc}", tag=f"wf{oc}")
            nc.gpsimd.dma_start(out=t, in_=wfeat[oc * 128 : (oc + 1) * 128, :])
            wf.append(t)
            t2 = const.tile([128, 2 * C], dt.float32, name=f"wo{oc}", tag=f"wo{oc}")
            nc.gpsimd.dma_start(out=t2, in_=wout[oc * 128 : (oc + 1) * 128, :])
            wo.append(t2)

        # BN params as [2, 128] (2 contiguous 512B descriptors each, instead
        # of 256 4-byte gather descriptors in [128, 2] layout).
        bnt = {}
        for key, hdl in bnp.items():
            t = const.tile([2, 128], dt.float32, name=f"t_{key}", tag=f"t_{key}")
            nc.gpsimd.dma_start(out=t, in_=hdl[:].rearrange("(t p) -> t p", p=128))
            bnt[key] = t


        # ---- BN scale/bias in [2, 128] layout, then one packed PE transpose:
        # s = gamma*rsqrt(var+eps), b = beta - mean*s; pack [s_f;b_f;s_o;b_o]
        # as [8, 128] -> transpose -> [128, 8] whose col 2q+t is chunk t.
        # Emitted mid-stream (i==3) so the ACT/DVE waits on the slow SWDGE
        # param loads don't block the f1T eviction pipeline. ----
        setup = ctx.enter_context(tc.tile_pool(name="setup", bufs=1))
        bn_cols = setup.tile([128, 8], dt.float32, name="bn_cols")
        sf = bn_cols[:, 0:2]
        bf = bn_cols[:, 2:4]
        so = bn_cols[:, 4:6]
        bo = bn_cols[:, 6:8]

        def emit_bn_prep(ps_pool):
            # separate [2, 128] tiles (engine writes must start at partition 0)
            sb_rows = []
            for pre in ("f", "o"):
                s = setup.tile([2, 128], dt.float32, name=f"s_{pre}", tag=f"s_{pre}")
                b = setup.tile([2, 128], dt.float32, name=f"b_{pre}", tag=f"b_{pre}")
                tmp = setup.tile([2, 128], dt.float32, name=f"tmp_{pre}", tag=f"tmp_{pre}")
                nc.scalar.activation(
                    out=tmp, in_=bnt[f"bn_{pre}_var"], func=AF.Sqrt, bias=eps_t, scale=1.0
                )
                nc.vector.reciprocal(out=tmp, in_=tmp)
                nc.vector.tensor_mul(s, bnt[f"bn_{pre}_gamma"], tmp)
                nc.vector.tensor_mul(tmp, bnt[f"bn_{pre}_mean"], s)
                nc.vector.tensor_sub(b, bnt[f"bn_{pre}_beta"], tmp)
                sb_rows += [s, b]
            bn_ps_t = ps_pool.tile([128, 8], dt.float32, name="bn_ps_t", tag="bn_ps_t")
            for qi, rt in enumerate(sb_rows):
                nc.tensor.transpose(bn_ps_t[:, 2 * qi : 2 * qi + 2], rt, ident[0:2, 0:2])
            nc.vector.tensor_copy(bn_cols, bn_ps_t)

        alg = ctx.enter_context(tc.tile_pool(name="alg", bufs=1))

        def emit_rhs_g():
            # rhs = [diag(sf) Wf | bf] per c-chunk (feeds g_ext = mid @ rhs)
            for cc in range(2):
                r = alg.tile([128, C + 1], dt.float32, name=f"rhs_g{cc}", tag=f"rhs_g{cc}")
                nc.vector.tensor_scalar(
                    out=r[:, 0:C], in0=wf[cc], scalar1=sf[:, cc : cc + 1],
                    scalar2=None, op0=OP.mult,
                )
                nc.vector.tensor_copy(r[:, C : C + 1], bf[:, cc : cc + 1])
                rhs_g.append(r)

        # ident2: same values as ident, but its last column is rewritten with
        # a dependency on piece 3 (out = 0*piece + ident). The W transposes
        # consume ident2, which pins them late in the PE queue — the Tile
        # scheduler otherwise hoists them to the queue front, where their
        # wait on the slow SWDGE weight loads blocks the piece transposes.
        ident2 = const.tile([128, 128], dt.float32, name="ident2")

        def emit_ident2():
            nc.vector.tensor_copy(ident2, ident)
            nc.vector.scalar_tensor_tensor(
                out=ident2[:, 127:128], in0=F_t[0][3][:, 0:1].bitcast(dt.float32),
                scalar=0.0, in1=ident[:, 127:128], op0=OP.mult, op1=OP.add,
            )

        def emit_wT(w_ps_pool):
            # W2T[j][128, 256] and W1T[j][128, 256] via identity-matmul
            # transposes of W_out blocks (j = input-channel chunk); both share
            # one [128, 512] PSUM bank-tile per chunk.
            for jc in range(2):
                W_ps = w_ps_pool.tile([128, 2 * C], dt.float32, name="W_ps", tag="W_ps")
                W2T_ps = W_ps[:, 0:C]
                W1T_ps = W_ps[:, C : 2 * C]
                for oc in range(2):
                    nc.tensor.matmul(
                        W2T_ps[:, oc * 128 : (oc + 1) * 128],
                        lhsT=wo[oc][:, C + jc * 128 : C + (jc + 1) * 128],
                        rhs=ident2,
                        start=(oc == 0),
                        stop=(oc == 1),
                    )
                    nc.tensor.matmul(
                        W1T_ps[:, oc * 128 : (oc + 1) * 128],
                        lhsT=wo[oc][:, jc * 128 : (jc + 1) * 128],
                        rhs=ident2,
                        start=(oc == 0),
                        stop=(oc == 1),
                    )
                t = alg.tile([128, C], dt.float32, name=f"W2T{jc}", tag=f"W2T{jc}")
                nc.vector.tensor_copy(t, W2T_ps)
                W2T_sb.append(t)
                t1 = alg.tile([128, C], dt.float32, name=f"W1T{jc}", tag=f"W1T{jc}")
                nc.scalar.copy(t1, W1T_ps)
                W1T_sb.append(t1)

        rhs_g = []
        W2T_sb = []
        W1T_sb = []

        # ---- mid = fbu @ F^T via per-h PE transposes, accumulated in PSUM;
        # fbuT (exact binary masks) is the cheap stationary operand, the
        # transposed F blocks stream as the moving operand ----
        midT_sb = alg.tile([128, 6], dt.float32, name="midT_sb")
        with tc.tile_pool(name="midps", bufs=1, space="PSUM") as midps:
            mid_ps = midps.tile([3, C], dt.float32, name="mid_ps")
            with tc.tile_pool(name="tr_ps", bufs=5, space="PSUM") as tr_ps_pool, \
                 tc.tile_pool(name="f1T_pool", bufs=8) as f1T_pool, \
                 tc.tile_pool(name="w_ps_pool", bufs=1, space="PSUM") as w_ps_pool:
                GPP = 64 // NPIECE  # hp-groups per piece
                for i in range(NPIECE):
                    if i == 3:
                        with tc.tile_wait_until(0.018):
                            emit_ident2()
                            emit_bn_prep(w_ps_pool)
                            emit_rhs_g()
                            emit_wT(w_ps_pool)
                    for g in range(GPP):
                        hp = GPP * i + g
                        tps = tr_ps_pool.tile([128, 512], F32R, name="tps")
                        for q in range(4):
                            h = 2 * hp + q // 2
                            cc = q % 2
                            nc.tensor.transpose(
                                tps[:, q * 128 : (q + 1) * 128],
                                f_slice(cc, h * 128, 128),
                                ident_r,
                            )
                        f1T = f1T_pool.tile([128, 512], F32R, name="f1T")
                        if hp % 2 == 0:
                            nc.vector.tensor_copy(f1T, tps)
                        else:
                            nc.scalar.copy(f1T, tps)
                        for q2 in range(2):
                            h = 2 * hp + q2
                            nc.tensor.matmul(
                                mid_ps[:, :],
                                lhsT=fbuT[:, h, :],
                                rhs=f1T[:, q2 * 256 : (q2 + 1) * 256],
                                start=(h == 0),
                                stop=(h == 127),
                            )
            mid_sb = alg.tile([3, C], dt.float32, name="mid_sb")
            nc.vector.tensor_copy(mid_sb, mid_ps)

        # mid^T via two small PE transposes (evicted in parallel on DVE/ACT)
        with tc.tile_pool(name="mt_ps", bufs=1, space="PSUM") as mt_ps_pool:
            mT_ps = mt_ps_pool.tile([128, 8], dt.float32, name="mT_ps")
            for cc in range(2):
                nc.tensor.transpose(
                    mT_ps[:, 4 * cc : 4 * cc + 3],
                    mid_sb[:, cc * 128 : (cc + 1) * 128],
                    ident[0:3, 0:3],
                )
            nc.vector.tensor_copy(midT_sb[:, 0:3], mT_ps[:, 0:3])
            nc.scalar.copy(midT_sb[:, 3:6], mT_ps[:, 4:7])

        # ---- tiny algebra: {gext, S} -> WeffT (+W1T during eviction), beff.
        # Dummy transposes between stages keep the PE p-state up while the
        # evictions round-trip through DVE/ACT. ----
        with tc.tile_pool(name="warm_psp", bufs=1, space="PSUM") as warm_ps_pool, \
             tc.tile_pool(name="alg_ps", bufs=1, space="PSUM") as alg_ps:
            warm_ps = warm_ps_pool.tile([128, 128], dt.float32, name="warm_ps")

            def keep_warm(src_ap, n):
                # cheap transposes whose input depends on a just-evicted tile,
                # so the Tile scheduler cannot hoist them early; they keep the
                # PE p-state up while the next stage's eviction round-trips
                for _ in range(n):
                    nc.tensor.transpose(warm_ps[:, 0:3], src_ap, ident[0:3, 0:3])

            keep_warm(mid_sb[0:3, 0:128], 3)
            # gext = mid @ [diag(sf) Wf | bf] : [3, C+1]; col C is u
            gext_ps = alg_ps.tile([3, C + 1], dt.float32, name="gext_ps", tag="gext_ps")
            for cc in range(2):
                nc.tensor.matmul(
                    gext_ps,
                    lhsT=midT_sb[:, cc * 3 : (cc + 1) * 3],
                    rhs=rhs_g[cc],
                    start=(cc == 0),
                    stop=(cc == 1),
                )
            # S = mid @ W2^T : [3, C]
            S_ps = alg_ps.tile([3, C], dt.float32, name="S_ps", tag="S_ps")
            for jc in range(2):
                nc.tensor.matmul(
                    S_ps,
                    lhsT=midT_sb[:, jc * 3 : (jc + 1) * 3],
                    rhs=W2T_sb[jc],
                    start=(jc == 0),
                    stop=(jc == 1),
                )
            gext_sb = alg.tile([3, C + 1], dt.float32, name="gext_sb")
            nc.vector.tensor_copy(gext_sb, gext_ps)
            S_sb = alg.tile([3, C], dt.float32, name="S_sb")
            nc.scalar.copy(S_sb, S_ps)
            keep_warm(gext_sb[0:3, 0:128], 4)

            # WeffT[cc] = W1T[cc] + g^T @ S (single K=3 matmul per c-chunk;
            # the W1T add happens during the PSUM eviction)
            WeffT_sb = []
            Wt_ps = []
            for cc in range(2):
                p = alg_ps.tile([128, C], dt.float32, name=f"Wt_ps{cc}", tag=f"Wt_ps{cc}")
                nc.tensor.matmul(
                    p, lhsT=gext_sb[:, cc * 128 : (cc + 1) * 128], rhs=S_sb,
                    start=True, stop=True,
                )
                Wt_ps.append(p)
            # beff = so * (S^T @ u) + bo  (off the first-final-matmul path)
            beff = alg.tile([128, 2], dt.float32, name="beff")
            wv_ps = []
            for oc in range(2):
                p = alg_ps.tile([128, 1], dt.float32, name=f"wv_ps{oc}", tag=f"wv_ps{oc}")
                nc.tensor.matmul(
                    p, lhsT=S_sb[:, oc * 128 : (oc + 1) * 128],
                    rhs=gext_sb[:, C : C + 1],
                    start=True, stop=True,
                )
                wv_ps.append(p)
            for cc in range(2):
                t = alg.tile([128, C], F32R, name=f"WeffT{cc}", tag=f"WeffT{cc}")
                nc.vector.tensor_tensor(t, Wt_ps[cc], W1T_sb[cc], OP.add)
                WeffT_sb.append(t)
            for oc in range(2):
                nc.vector.tensor_scalar(
                    out=beff[:, oc : oc + 1], in0=wv_ps[oc],
                    scalar1=so[:, oc : oc + 1], scalar2=bo[:, oc : oc + 1],
                    op0=OP.mult, op1=OP.add,
                )

        # ---- final: out = so * (Weff @ F) + beff, streamed over n ----
        # 2-bank PSUM super-tiles: 4 matmuls (2 n-halves x 2 c-chunks). Two
        # super-tiles pair into one [128, 2048] SBUF tile whose halves evict
        # on DVE and ACT in parallel, then one 1 MB store on the sync ring.
        NT = 512
        with tc.tile_pool(name="fin_ps", bufs=4, space="PSUM") as fin_ps, \
             tc.tile_pool(name="osb", bufs=4) as osb_pool:
            for oc in range(2):
                for gg in range(HW // (4 * NT)):
                    ot = osb_pool.tile([128, 4 * NT], dt.float32, name="ot")
                    for half in range(2):
                        ps2 = fin_ps.tile([128, 2 * NT], dt.float32, name="ps2")
                        for cc in range(2):
                            for t in range(2):
                                nt = 4 * gg + 2 * half + t
                                nc.tensor.matmul(
                                    ps2[:, t * NT : (t + 1) * NT],
                                    lhsT=WeffT_sb[cc][:, oc * 128 : (oc + 1) * 128],
                                    rhs=f_slice(cc, nt * NT, NT),
                                    start=(cc == 0),
                                    stop=(cc == 1),
                                )
                        dst = ot[:, half * 2 * NT : (half + 1) * 2 * NT]
                        if half == 0:
                            nc.vector.tensor_scalar(
                                out=dst, in0=ps2, scalar1=so[:, oc : oc + 1],
                                scalar2=beff[:, oc : oc + 1], op0=OP.mult, op1=OP.add,
                            )
                        else:
                            nc.scalar.activation(
                                out=dst, in_=ps2, func=AF.Identity,
                                bias=beff[:, oc : oc + 1], scale=so[:, oc : oc + 1],
                            )
                    nc.sync.dma_start(
                        out=out_d[
                            oc * 128 : (oc + 1) * 128, 4 * gg * NT : (4 * gg + 4) * NT
                        ],
                        in_=ot,
                    )

    _split_drain_waits(nc)
    return nc


_NC_CACHE = None


def _get_nc():
    global _NC_CACHE
    if _NC_CACHE is None:
        _NC_CACHE = build_nc()
    return _NC_CACHE


def kernel(**inputs):
    feature = np.asarray(inputs["feature"], dtype=np.float32)
    m = np.asarray(inputs["m"], dtype=np.float32)
    shared = {}
    shared["w_feat"] = np.asarray(inputs["w_feat"], dtype=np.float32)
    shared["w_out"] = np.asarray(inputs["w_out"], dtype=np.float32)
    for pre in ("f", "o"):
        for nm in ("gamma", "beta", "mean", "var"):
            key = f"bn_{pre}_{nm}"
            shared[key] = np.asarray(inputs[key], dtype=np.float32)

    nc = _get_nc()
    in_maps = []
    for i in range(NCORES):
        im = dict(shared)
        im["feature"] = np.ascontiguousarray(feature[i].reshape(C, HW))
        im["m"] = np.ascontiguousarray(m[i].reshape(H, W))
        in_maps.append(im)

    res = run_bass_kernel_spmd(nc, in_maps, core_ids=list(range(NCORES)))
    out = np.stack([res.results[i]["out"].reshape(C, H, W) for i in range(NCORES)])
    return out
